# revision 8
# baseline (speedup 1.0000x reference)
"""Trainium2 Bass kernel for CustomizablePatchDominantGradientOrientation.

Pipeline per patch (32x32, fp32):
  sobel (replicate pad, [1,2,1]x[-1,0,1] separable; /8 dropped - the final
  angle is invariant to a global scale on (gx, gy, mag))
  mag = sqrt(gx^2+gy^2+eps'), theta = 2*atan(gy/(mag+gx))  (half-angle atan2)
  soft 36-bin histogram of theta weighted by mag  -> 36 fused custom-DVE
  passes (relu(min(a-c0, c1-a))*mag with free-axis accumulate)
  circular [w0,w1,w2] smoothing, argmax, parabolic refinement -> angle.

Data parallel: B=32768 patches sharded over 8 NeuronCores (4096 each);
per core 32 tiles of [128 patches x 1024 pixels].  Layout is patch-major:
partitions = patches, free axis = pixels.
"""

import math

import numpy as np

NBINS = 36
PI = math.pi
PATCH = 32
HW = PATCH * PATCH
P = 128          # partitions (patches per tile)
N_CORES = 8
GROUP = 4        # tiles per ACT-table-set phase group

_BUILD_CACHE = {}
_OPS_REGISTERED = {}


# --------------------------------------------------------------------------
# custom DVE ops
# --------------------------------------------------------------------------
def _register_custom_ops():
    """Register the fused histogram ops at runtime (row assignment + sha pin,
    exactly what a source-level `OPS.append` would do)."""
    if _OPS_REGISTERED:
        return _OPS_REGISTERED
    from operator import add as _op_add

    import concourse.dve_ops as dve_ops
    from concourse.dve_ops import DveOp
    from concourse.dve_spec import (
        Spec, Src0, Src1, C0, C1, Zero, relu, minn, maxx, lower, _has_src1,
    )
    from concourse.dve_uop import DveOpSpec

    def _reg(name, spec):
        if name in dve_ops._SUB_OPCODE_FOR_NAME:
            for op in dve_ops.OPS:
                if op.name == name:
                    return op
        row = dve_ops._CUSTOM_DVE_ROW_BASE + len(dve_ops.OPS)
        assert row < 0x20, "custom-DVE row budget exhausted"
        dve_ops._SUB_OPCODE_FOR_NAME[name] = row
        shas = {}
        for ver in ("v3", "v4"):
            s = DveOpSpec(name=name, opcode=row, uops=lower(spec, ver=ver),
                          rd1_en=_has_src1(spec))
            shas[ver] = s.sha(ver)
        op = DveOp(name, spec, subdim=False, uops_sha=shas)
        dve_ops.OPS.append(op)
        dve_ops.CUSTOM_DVE_SPECS[name] = spec
        return op

    def _tri_ref(in0, in1, s0, s1, imm2):
        b = np.maximum(np.minimum(in0 - s0, s1 - in0), 0.0).astype(np.float32) * in1
        return b, b.reshape(b.shape[0], -1).sum(axis=-1, keepdims=True)

    def _wrap_ref(in0, in1, s0, s1, imm2):
        b = np.maximum(np.maximum(s0 - in0, in0 - s1), 0.0).astype(np.float32) * in1
        return b, b.reshape(b.shape[0], -1).sum(axis=-1, keepdims=True)

    def _avg_ref(in0, in1, s0, s1, imm2):
        return ((in0 + in1) * s0).astype(np.float32)

    _OPS_REGISTERED["tri"] = _reg(
        "HIST_TRI_ANT",
        Spec(body=relu(minn(Src0 - C0, C1 - Src0)) * Src1,
             accum=_op_add, accum_init=Zero, reference=_tri_ref))
    _OPS_REGISTERED["wrap"] = _reg(
        "HIST_WRAP_ANT",
        Spec(body=relu(maxx(C0 - Src0, Src0 - C1)) * Src1,
             accum=_op_add, accum_init=Zero, reference=_wrap_ref))
    _OPS_REGISTERED["avg"] = _reg(
        "AVG_SCALED_ANT",
        Spec(body=(Src0 + Src1) * C0, reference=_avg_ref))
    return _OPS_REGISTERED


# --------------------------------------------------------------------------
# kernel build
# --------------------------------------------------------------------------
def _build(b_core, smooth_w, wk_is_ones):
    import concourse.bacc as bacc
    import concourse.mybir as mybir
    from concourse.tile import TileContext
    from concourse.bass import broadcast_tensor_aps

    ops = _register_custom_ops()
    TRI, WRAP, AVG = ops["tri"], ops["wrap"], ops["avg"]

    f32 = mybir.dt.float32
    Alu = mybir.AluOpType
    Act = mybir.ActivationFunctionType

    n_tiles = b_core // P
    assert b_core % P == 0
    w0, w1, w2 = (float(x) for x in smooth_w)

    nc = bacc.Bacc(None, target_bir_lowering=False, debug=False)
    patch_in = nc.dram_tensor("patch", [b_core, HW], f32, kind="ExternalInput")
    # consts: iota36 repeated n_tiles times, then (iota36 - 64) repeated
    consts_in = nc.dram_tensor("consts", [P, 2 * n_tiles * NBINS], f32,
                               kind="ExternalInput")
    wk_in = None
    if not wk_is_ones:
        wk_in = nc.dram_tensor("wk", [P, HW], f32, kind="ExternalInput")
    out_t = nc.dram_tensor("angle", [b_core], f32, kind="ExternalOutput")

    # per-bin tri constants in atan units: t = (36/pi)*a + 18
    # bin k (k>=1): c0=(k-19)*pi/36, c1=(k-17)*pi/36
    # bin 0 wrap:   relu(max(c0 - a, a - c1)) with c0=-17pi/36, c1=17pi/36
    def c_lo(k):
        return (k - 19.0) * PI / 36.0

    def c_hi(k):
        return (k - 17.0) * PI / 36.0

    with TileContext(nc) as tc:
        with tc.tile_pool(name="pool", bufs=2) as pool, \
             tc.tile_pool(name="persist", bufs=1) as pp:
            IOTA = pp.tile([P, n_tiles, NBINS], f32)
            IOTA64 = pp.tile([P, n_tiles, NBINS], f32)
            nc.sync.dma_start(IOTA[:], consts_in[:, 0:n_tiles * NBINS])
            nc.sync.dma_start(IOTA64[:], consts_in[:, n_tiles * NBINS:])
            WK = None
            if wk_in is not None:
                WK = pp.tile([P, HW], f32)
                nc.sync.dma_start(WK[:], wk_in[:])

            HEXT = pp.tile([P, n_tiles, NBINS + 2], f32)
            ANG = pp.tile([P, n_tiles], f32)

            n_groups = (n_tiles + GROUP - 1) // GROUP
            for g in range(n_groups):
                tiles = range(g * GROUP, min((g + 1) * GROUP, n_tiles))
                slot = {}
                # ---- phase A: sobel, magnitude (sqrt table set) ----
                for t in tiles:
                    s = t % GROUP
                    X = pool.tile([P, HW], f32, tag="x", name=f"x{t}")
                    nc.sync.dma_start(X[:], patch_in[t * P:(t + 1) * P, :])
                    X3 = X.rearrange("p (r c) -> p r c", c=PATCH)

                    SV = pool.tile([P, HW], f32, tag="sv", name=f"sv{t}")
                    # vertical [1,2,1] with replicate rows
                    nc.vector.scalar_tensor_tensor(
                        out=SV[:, 32:992], in0=X[:, 32:992], scalar=2.0,
                        in1=X[:, 0:960], op0=Alu.mult, op1=Alu.add)
                    nc.vector.tensor_tensor(
                        SV[:, 32:992], SV[:, 32:992], X[:, 64:1024], Alu.add)
                    nc.vector.scalar_tensor_tensor(
                        out=SV[:, 0:32], in0=X[:, 0:32], scalar=3.0,
                        in1=X[:, 32:64], op0=Alu.mult, op1=Alu.add)
                    nc.vector.scalar_tensor_tensor(
                        out=SV[:, 992:1024], in0=X[:, 992:1024], scalar=3.0,
                        in1=X[:, 960:992], op0=Alu.mult, op1=Alu.add)
                    SV3 = SV.rearrange("p (r c) -> p r c", c=PATCH)

                    GX = pool.tile([P, HW], f32, tag=f"gx{s}", bufs=1,
                                   name=f"gx{t}")
                    GX3 = GX.rearrange("p (r c) -> p r c", c=PATCH)
                    # horizontal central difference with replicate cols
                    nc.vector.tensor_tensor(
                        GX3[:, :, 1:31], SV3[:, :, 2:32], SV3[:, :, 0:30],
                        Alu.subtract)
                    nc.vector.tensor_tensor(
                        GX3[:, :, 0:1], SV3[:, :, 1:2], SV3[:, :, 0:1],
                        Alu.subtract)
                    nc.vector.tensor_tensor(
                        GX3[:, :, 31:32], SV3[:, :, 31:32], SV3[:, :, 30:31],
                        Alu.subtract)

                    SH = pool.tile([P, HW], f32, tag="sh", name=f"sh{t}")
                    SH3 = SH.rearrange("p (r c) -> p r c", c=PATCH)
                    # horizontal [1,2,1] with replicate cols
                    nc.vector.scalar_tensor_tensor(
                        out=SH3[:, :, 1:31], in0=X3[:, :, 1:31], scalar=2.0,
                        in1=X3[:, :, 0:30], op0=Alu.mult, op1=Alu.add)
                    nc.vector.tensor_tensor(
                        SH3[:, :, 1:31], SH3[:, :, 1:31], X3[:, :, 2:32],
                        Alu.add)
                    nc.vector.scalar_tensor_tensor(
                        out=SH3[:, :, 0:1], in0=X3[:, :, 0:1], scalar=3.0,
                        in1=X3[:, :, 1:2], op0=Alu.mult, op1=Alu.add)
                    nc.vector.scalar_tensor_tensor(
                        out=SH3[:, :, 31:32], in0=X3[:, :, 31:32], scalar=3.0,
                        in1=X3[:, :, 30:31], op0=Alu.mult, op1=Alu.add)

                    GY = pool.tile([P, HW], f32, tag=f"gy{s}", bufs=1,
                                   name=f"gy{t}")
                    # vertical central difference with replicate rows
                    nc.vector.tensor_tensor(
                        GY[:, 32:992], SH[:, 64:1024], SH[:, 0:960],
                        Alu.subtract)
                    nc.vector.tensor_tensor(
                        GY[:, 0:32], SH[:, 32:64], SH[:, 0:32], Alu.subtract)
                    nc.vector.tensor_tensor(
                        GY[:, 992:1024], SH[:, 992:1024], SH[:, 960:992],
                        Alu.subtract)

                    if WK is not None:
                        nc.vector.tensor_tensor(GX[:], GX[:], WK[:], Alu.mult)
                        nc.vector.tensor_tensor(GY[:], GY[:], WK[:], Alu.mult)

                    # g2 = gx^2 + gy^2 + eps  (eps scaled by 8^2 vs reference)
                    # sv/sh slots are dead here; reuse their tags for squares.
                    # Exact fp32 multiplies on GPSIMD (ACT Square is ~1e-5
                    # off, which poisons the magnitude beyond repair).
                    X2 = pool.tile([P, HW], f32, tag="sv", name=f"x2{t}")
                    Y2 = pool.tile([P, HW], f32, tag="sh", name=f"y2{t}")
                    nc.gpsimd.tensor_tensor(X2[:], GX[:], GX[:], Alu.mult)
                    nc.gpsimd.tensor_tensor(Y2[:], GY[:], GY[:], Alu.mult)
                    G2 = pool.tile([P, HW], f32, tag="g2", name=f"g2{t}")
                    nc.vector.scalar_tensor_tensor(
                        out=G2[:], in0=X2[:], scalar=6.4e-17, in1=Y2[:],
                        op0=Alu.add, op1=Alu.add)
                    M = pool.tile([P, HW], f32, tag=f"m{s}", bufs=1,
                                   name=f"m{t}")
                    nc.scalar.activation(M[:], G2[:], Act.Sqrt)
                    # one Newton step: m = 0.5*(y0 + g2/y0)
                    RC = pool.tile([P, HW], f32, tag="rc", name=f"rc{t}")
                    SC = pool.tile([P, HW], f32, tag="sc", name=f"sc{t}")
                    nc.vector.reciprocal_approx_accurate(RC[:], M[:], SC[:])
                    nc.vector.tensor_tensor(SC[:], G2[:], RC[:], Alu.mult)
                    nc.vector._custom_dve(AVG, out=M[:], in0=SC[:], in1=M[:],
                                          s0=0.5)
                    slot[t] = (GX, GY, M)

                # ---- phase B: orientation + histogram (sigmoid table set) --
                for t in tiles:
                    GX, GY, M = slot[t]
                    # d = max(m + gx, 1e-30): the clamp both avoids the
                    # recip(0)=NaN edge and pins rounding-negative d to the
                    # correct wrap side (t -> 36/0 by sign of gy).
                    D = pool.tile([P, HW], f32, tag="g2", name=f"d{t}")
                    nc.vector.tensor_tensor(D[:], M[:], GX[:], Alu.add)
                    nc.vector.tensor_scalar(D[:], D[:], 1e-30, None, Alu.max)
                    RC = pool.tile([P, HW], f32, tag="rc", name=f"rcb{t}")
                    SC = pool.tile([P, HW], f32, tag="sc", name=f"scb{t}")
                    nc.vector.reciprocal_approx_accurate(RC[:], D[:], SC[:])
                    nc.vector.tensor_tensor(SC[:], GY[:], RC[:], Alu.mult)
                    A = pool.tile([P, HW], f32, tag="a", name=f"a{t}")
                    nc.scalar.activation(A[:], SC[:], Act.Arctan)

                    SCR = pool.tile([P, HW], f32, tag="scr", name=f"scr{t}")
                    for k in range(NBINS):
                        acc = HEXT[:, t, k + 1:k + 2]
                        if k == 0:
                            nc.vector._custom_dve(
                                WRAP, out=SCR[:], in0=A[:], in1=M[:],
                                s0=-17.0 * PI / 36.0, s1=17.0 * PI / 36.0,
                                accum_out=acc)
                        else:
                            nc.vector._custom_dve(
                                TRI, out=SCR[:], in0=A[:], in1=M[:],
                                s0=c_lo(k), s1=c_hi(k), accum_out=acc)

            # ---- tail: smoothing, argmax, refinement (batched) ----
            nc.vector.tensor_copy(HEXT[:, :, 0:1], HEXT[:, :, 36:37])
            nc.vector.tensor_copy(HEXT[:, :, 37:38], HEXT[:, :, 1:2])

            SM = pp.tile([P, n_tiles, NBINS], f32)
            nc.vector.tensor_scalar(SM[:], HEXT[:, :, 2:38], w2, None,
                                    Alu.mult)
            nc.vector.scalar_tensor_tensor(
                out=SM[:], in0=HEXT[:, :, 0:36], scalar=w0, in1=SM[:],
                op0=Alu.mult, op1=Alu.add)
            HS = pp.tile([P, n_tiles, NBINS], f32)
            nc.vector.scalar_tensor_tensor(
                out=HS[:], in0=HEXT[:, :, 1:37], scalar=w1, in1=SM[:],
                op0=Alu.mult, op1=Alu.add)

            VMAX = pp.tile([P, n_tiles, 1], f32)
            nc.vector.tensor_reduce(VMAX[:], HS[:], mybir.AxisListType.X,
                                    Alu.max)
            EQ = pp.tile([P, n_tiles, NBINS], f32)
            hs_b, vmax_b = broadcast_tensor_aps(HS[:], VMAX[:])
            nc.vector.tensor_tensor(EQ[:], hs_b, vmax_b, Alu.is_equal)
            nc.vector.tensor_tensor(EQ[:], EQ[:], IOTA64[:], Alu.mult)
            IDX = pp.tile([P, n_tiles, 1], f32)
            nc.vector.tensor_reduce(IDX[:], EQ[:], mybir.AxisListType.X,
                                    Alu.min)
            nc.vector.tensor_scalar(IDX[:], IDX[:], 64.0, None, Alu.add)

            def neighbor_value(shift, wrap_thr, wrap_add, nm):
                IDXN = pp.tile([P, n_tiles, 1], f32, name=f"idxn_{nm}")
                nc.vector.tensor_scalar(IDXN[:], IDX[:], float(shift), None,
                                        Alu.add)
                WADJ = pp.tile([P, n_tiles, 1], f32, name=f"wadj_{nm}")
                if wrap_add < 0:
                    nc.vector.tensor_scalar(WADJ[:], IDXN[:], wrap_thr,
                                            float(wrap_add), Alu.is_gt,
                                            Alu.mult)
                else:
                    nc.vector.tensor_scalar(WADJ[:], IDXN[:], wrap_thr,
                                            float(wrap_add), Alu.is_lt,
                                            Alu.mult)
                nc.vector.tensor_tensor(IDXN[:], IDXN[:], WADJ[:], Alu.add)
                DIF = pp.tile([P, n_tiles, NBINS], f32, name=f"dif_{nm}")
                iota_b, idxn_b = broadcast_tensor_aps(IOTA[:], IDXN[:])
                nc.vector.tensor_tensor(DIF[:], iota_b, idxn_b, Alu.subtract)
                nc.vector.tensor_scalar(DIF[:], DIF[:], 0.0, None,
                                        Alu.is_equal)
                nc.vector.tensor_tensor(DIF[:], DIF[:], HS[:], Alu.mult)
                V = pp.tile([P, n_tiles, 1], f32, name=f"v_{nm}")
                nc.vector.tensor_reduce(V[:], DIF[:], mybir.AxisListType.X,
                                        Alu.add)
                return V

            VP = neighbor_value(+1, 35.5, -36.0, "p")
            VM = neighbor_value(-1, -0.5, +36.0, "m")

            NUM = pp.tile([P, n_tiles, 1], f32)
            nc.vector.tensor_tensor(NUM[:], VP[:], VM[:], Alu.subtract)
            SUMN = pp.tile([P, n_tiles, 1], f32)
            nc.vector.tensor_tensor(SUMN[:], VP[:], VM[:], Alu.add)
            DEN = pp.tile([P, n_tiles, 1], f32)
            nc.vector.tensor_scalar(DEN[:], VMAX[:], 2.0, None, Alu.mult)
            nc.vector.tensor_tensor(DEN[:], DEN[:], SUMN[:], Alu.subtract)
            RECD = pp.tile([P, n_tiles, 1], f32)
            SCD = pp.tile([P, n_tiles, 1], f32)
            nc.vector.reciprocal_approx_accurate(RECD[:], DEN[:], SCD[:])
            REF = pp.tile([P, n_tiles, 1], f32)
            nc.vector.scalar_tensor_tensor(
                out=REF[:], in0=NUM[:], scalar=0.5, in1=RECD[:],
                op0=Alu.mult, op1=Alu.mult)
            nc.vector.tensor_tensor(REF[:], IDX[:], REF[:], Alu.add)
            nc.vector.tensor_scalar(ANG[:], REF[:, :, 0], -2.0 * PI / NBINS,
                                    PI, Alu.mult, Alu.add)

            out_view = out_t[:].rearrange("(t p) -> p t", p=P)
            nc.sync.dma_start(out_view, ANG[:])

    nc.compile()
    return nc


def _get_built(b_core, smooth_w, wk_is_ones):
    key = (b_core, tuple(float(x) for x in smooth_w), bool(wk_is_ones))
    if key not in _BUILD_CACHE:
        _BUILD_CACHE[key] = _build(b_core, smooth_w, wk_is_ones)
    return _BUILD_CACHE[key]


# --------------------------------------------------------------------------
# host entry point
# --------------------------------------------------------------------------
def kernel(patch, weight_kernel, smooth_w):
    from concourse import bass_utils

    patch = np.ascontiguousarray(np.asarray(patch, dtype=np.float32))
    weight_kernel = np.asarray(weight_kernel, dtype=np.float32)
    smooth_w = np.asarray(smooth_w, dtype=np.float32)

    B = patch.shape[0]
    assert B % (N_CORES * P) == 0, f"B={B} not divisible by {N_CORES * P}"
    b_core = B // N_CORES
    n_tiles = b_core // P

    wk_is_ones = bool(np.all(weight_kernel == 1.0))
    nc = _get_built(b_core, smooth_w, wk_is_ones)

    x = patch.reshape(N_CORES, b_core, HW)

    iota = np.tile(np.arange(NBINS, dtype=np.float32), n_tiles)
    consts_row = np.concatenate([iota, iota - 64.0]).astype(np.float32)
    consts = np.ascontiguousarray(
        np.broadcast_to(consts_row, (P, consts_row.size)))

    in_maps = []
    for i in range(N_CORES):
        m = {"patch": np.ascontiguousarray(x[i]), "consts": consts}
        if not wk_is_ones:
            m["wk"] = np.ascontiguousarray(
                np.broadcast_to(weight_kernel.reshape(-1), (P, HW)))
        in_maps.append(m)

    res = bass_utils.run_bass_kernel_spmd(nc, in_maps,
                                          core_ids=list(range(N_CORES)))
    out = np.concatenate([r["angle"] for r in res.results])
    return out.astype(np.float32)
